# revision 11
# baseline (speedup 1.0000x reference)
"""KAN layer (identity edges) Trainium2 kernel.

output[b, o] = sum_i x[b, i]  for all o  -- row-sum broadcast to (B, 1024).

Data-parallel over 8 NeuronCores: each core gets 8192 rows of x
(65536 x 1024 f32), computes row sums on the Vector engine, broadcasts
across the feature dim on the Scalar (Activation) engine, and DMAs the
full (8192, 1024) shard out.

The kernel is HBM-bound (32MB in + 32MB out per core at ~430 GB/s
aggregate): input DMAs are dispatched on the SP (sync) HWDGE queue and
output DMAs on the Activation (scalar) HWDGE queue so the two streams
never head-of-line block each other, with reduce (vector) and broadcast
(scalar) on separate engines so neither stalls the out stream. R=4 rows
per tile keeps the pipeline fine-grained enough that the out stream
ramps early and the drain tail after the last input lands stays short,
without paying R=2's extra per-iteration semaphore/dispatch overhead.

Layout: partition p owns 64 consecutive DRAM rows (rearrange
"(p n) d -> p n d"), so each DMA moves R*4KB contiguous bytes per
partition (8KB descriptors, full DMA-engine rate).
"""

import numpy as np

import concourse.tile as tile
from concourse import bacc, mybir
from concourse.bass_utils import run_bass_kernel_spmd

N_CORES = 8
BATCH = 65536
FEAT = 1024
ROWS = BATCH // N_CORES        # 8192 rows per core
P = 128                        # SBUF partitions
ROWS_PER_PART = ROWS // P      # 64 consecutive rows owned by each partition

_nc_cache = []


def _build(
    R=8,
    in_bufs=2,
    out_bufs=2,
    dma_engine="gpsimd",
    in_dma=None,
    out_dma=None,
    inplace=False,
    bcast_engine="vector",
    sum_bufs=4,
):
    """in_dma / out_dma: engine name or comma-separated list cycled per iter.
    bcast_engine: engine name or comma-separated list cycled per iter.
    R: rows-per-partition per tile — int, or list of ints summing to
    ROWS_PER_PART (a taper schedule)."""
    if isinstance(R, int):
        schedule = [R] * (ROWS_PER_PART // R)
    else:
        schedule = list(R)
        assert sum(schedule) == ROWS_PER_PART, schedule
    nc = bacc.Bacc()
    x = nc.declare_dram_parameter("x", [ROWS, FEAT], mybir.dt.float32, isOutput=False)
    y = nc.declare_dram_parameter("y", [ROWS, FEAT], mybir.dt.float32, isOutput=True)
    xv = x[:, :].rearrange("(p n) d -> p n d", p=P)
    yv = y[:, :].rearrange("(p n) d -> p n d", p=P)

    def engines(spec):
        return [getattr(nc, n) for n in spec.split(",")]

    dmas_in = engines(in_dma or dma_engine)
    dmas_out = engines(out_dma or dma_engine)
    bcasts = engines(bcast_engine)

    with tile.TileContext(nc) as tc:
        with (
            tc.tile_pool(name="inp", bufs=in_bufs) as inp,
            tc.tile_pool(name="outp", bufs=out_bufs) as outp,
            tc.tile_pool(name="sums", bufs=sum_bufs) as sums_pool,
        ):
            row = 0
            for i, r in enumerate(schedule):
                t = inp.tile([P, r, FEAT], mybir.dt.float32, name=f"t{i}", tag="t")
                dmas_in[i % len(dmas_in)].dma_start(
                    out=t[:, :, :], in_=xv[:, row : row + r, :]
                )

                s = sums_pool.tile([P, r], mybir.dt.float32, name=f"s{i}", tag="s")
                nc.vector.reduce_sum(
                    out=s[:, :], in_=t[:, :, :], axis=mybir.AxisListType.X
                )

                o = (
                    t
                    if inplace
                    else outp.tile([P, r, FEAT], mybir.dt.float32, name=f"o{i}", tag="o")
                )
                be = bcasts[i % len(bcasts)]
                copy_fn = getattr(be, "tensor_copy", None) or be.copy
                copy_fn(out=o[:, :, :], in_=s[:, :].to_broadcast([P, r, FEAT]))
                dmas_out[i % len(dmas_out)].dma_start(
                    out=yv[:, row : row + r, :], in_=o[:, :, :]
                )
                row += r
    nc.finalize()
    return nc


BEST = dict(
    R=4,
    in_bufs=6,
    out_bufs=6,
    in_dma="sync",
    out_dma="scalar",
    bcast_engine="scalar",
)


def _get_nc():
    if not _nc_cache:
        _nc_cache.append(_build(**BEST))
    return _nc_cache[0]


def kernel(x: np.ndarray) -> np.ndarray:
    nc = _get_nc()
    x = np.ascontiguousarray(np.asarray(x), dtype=np.float32)
    shards = np.split(x, N_CORES, axis=0)
    in_maps = [{"x": s} for s in shards]
    res = run_bass_kernel_spmd(nc, in_maps, list(range(N_CORES)))
    return np.concatenate([res.results[i]["y"] for i in range(N_CORES)], axis=0)



# revision 12
# speedup vs baseline: 1.1535x; 1.1535x over previous
"""KAN layer (identity edges) Trainium2 kernel.

output[b, o] = sum_i x[b, i]  for all o  -- row-sum broadcast to (B, 1024).

Data-parallel over 8 NeuronCores: each core gets 8192 rows of x
(65536 x 1024 f32), computes row sums on the Vector engine, broadcasts
across the feature dim on the Scalar (Activation) engine, and DMAs the
full (8192, 1024) shard out.

The kernel is HBM-bound (32MB in + 32MB out per core at ~430 GB/s
aggregate): input DMAs are dispatched on the SP (sync) HWDGE queue and
output DMAs on the Activation (scalar) HWDGE queue so the two streams
never head-of-line block each other, with reduce (vector) and broadcast
(scalar) on separate engines so neither stalls the out stream. R=4 rows
per tile keeps the pipeline fine-grained enough that the out stream
ramps early and the drain tail after the last input lands stays short,
without paying R=2's extra per-iteration semaphore/dispatch overhead.

Layout: partition p owns 64 consecutive DRAM rows (rearrange
"(p n) d -> p n d"), so each DMA moves R*4KB contiguous bytes per
partition (8KB descriptors, full DMA-engine rate).
"""

import os

# Whole-tile dependency tracking only: this kernel always consumes whole
# tiles, and the coarser semaphore structure shortens the kernel's
# launch/drain choreography measurably (~169us vs ~195us median).
os.environ.setdefault("BY_DEFAULT_DISABLE_SUBTILE_DEPS", "1")

import numpy as np

import concourse.tile as tile
from concourse import bacc, mybir
from concourse.bass_utils import run_bass_kernel_spmd

N_CORES = 8
BATCH = 65536
FEAT = 1024
ROWS = BATCH // N_CORES        # 8192 rows per core
P = 128                        # SBUF partitions
ROWS_PER_PART = ROWS // P      # 64 consecutive rows owned by each partition

_nc_cache = []


def _build(
    R=8,
    in_bufs=2,
    out_bufs=2,
    dma_engine="gpsimd",
    in_dma=None,
    out_dma=None,
    inplace=False,
    bcast_engine="vector",
    sum_bufs=4,
):
    """in_dma / out_dma: engine name or comma-separated list cycled per iter.
    bcast_engine: engine name or comma-separated list cycled per iter.
    R: rows-per-partition per tile — int, or list of ints summing to
    ROWS_PER_PART (a taper schedule)."""
    if isinstance(R, int):
        schedule = [R] * (ROWS_PER_PART // R)
    else:
        schedule = list(R)
        assert sum(schedule) == ROWS_PER_PART, schedule
    nc = bacc.Bacc()
    x = nc.declare_dram_parameter("x", [ROWS, FEAT], mybir.dt.float32, isOutput=False)
    y = nc.declare_dram_parameter("y", [ROWS, FEAT], mybir.dt.float32, isOutput=True)
    xv = x[:, :].rearrange("(p n) d -> p n d", p=P)
    yv = y[:, :].rearrange("(p n) d -> p n d", p=P)

    def engines(spec):
        return [getattr(nc, n) for n in spec.split(",")]

    dmas_in = engines(in_dma or dma_engine)
    dmas_out = engines(out_dma or dma_engine)
    bcasts = engines(bcast_engine)

    with tile.TileContext(nc) as tc:
        with (
            tc.tile_pool(name="inp", bufs=in_bufs) as inp,
            tc.tile_pool(name="outp", bufs=out_bufs) as outp,
            tc.tile_pool(name="sums", bufs=sum_bufs) as sums_pool,
        ):
            row = 0
            for i, r in enumerate(schedule):
                t = inp.tile([P, r, FEAT], mybir.dt.float32, name=f"t{i}", tag="t")
                dmas_in[i % len(dmas_in)].dma_start(
                    out=t[:, :, :], in_=xv[:, row : row + r, :]
                )

                s = sums_pool.tile([P, r], mybir.dt.float32, name=f"s{i}", tag="s")
                nc.vector.reduce_sum(
                    out=s[:, :], in_=t[:, :, :], axis=mybir.AxisListType.X
                )

                o = (
                    t
                    if inplace
                    else outp.tile([P, r, FEAT], mybir.dt.float32, name=f"o{i}", tag="o")
                )
                be = bcasts[i % len(bcasts)]
                copy_fn = getattr(be, "tensor_copy", None) or be.copy
                copy_fn(out=o[:, :, :], in_=s[:, :].to_broadcast([P, r, FEAT]))
                dmas_out[i % len(dmas_out)].dma_start(
                    out=yv[:, row : row + r, :], in_=o[:, :, :]
                )
                row += r
    nc.finalize()
    return nc


BEST = dict(
    R=4,
    in_bufs=6,
    out_bufs=6,
    in_dma="sync",
    out_dma="scalar",
    bcast_engine="scalar",
)


def _get_nc():
    if not _nc_cache:
        _nc_cache.append(_build(**BEST))
    return _nc_cache[0]


def kernel(x: np.ndarray) -> np.ndarray:
    nc = _get_nc()
    x = np.ascontiguousarray(np.asarray(x), dtype=np.float32)
    shards = np.split(x, N_CORES, axis=0)
    in_maps = [{"x": s} for s in shards]
    res = run_bass_kernel_spmd(nc, in_maps, list(range(N_CORES)))
    return np.concatenate([res.results[i]["y"] for i in range(N_CORES)], axis=0)

